# revision 43
# baseline (speedup 1.0000x reference)
"""Trainium2 Bass kernel for nn_AndAttention.

B=16384 rows; per row: 2-token self-attention over (x1,x2) [D=1024 each],
concat -> h [2048], then 4x (Linear(2048,2048)+ReLU) and Linear(2048,1024).

Sharding: data-parallel over batch across 8 NeuronCores (2048 rows/core),
weights replicated. No collectives.

Layout strategy (per core):
  - Activations feature-major in SBUF, chunk-major slots: h[P, 8 chunk,
    2 tok, batch]; feature f = tok*1024 + chunk*128 + i lives in slot
    (chunk, tok). Weight k-axes are permuted on the host to match.
  - 2-token softmax via the delta trick: with delta = x1-x2,
      d0 = x1.delta/32 = s11-s12   (DVE stt with accumulate)
      e  = |delta|^2/32            (ACT Square with accumulate)
      d1 = d0 - e = s12-s22
      a00 = sigmoid(d0), a10 = sigmoid(d1)
    and y0 = x2 + a00*delta, y1 = x2 + a10*delta.
  - Attention combine+transpose fused on the PE with only 2 stationary
    operands per chunk (x2 and delta):
      psum[f, 0:256] = x2c.T @ [I|I] + dc.T @ [diag(a00)|diag(a10)]
    Two chunks share one full PSUM bank (2 accumulation groups per bank),
    evicted in a single [128,512] copy per pair (ACT/DVE alternating).
  - Attention tiles are interleaved per-tile into layer 1's m-loop via
    hooks so stats/DMA always overlap PE work and HAM stays warm.
  - Warmup matmuls + ACT table preloads (Square, Sigmoid) at t=0.
  - MLP layers: lhsT = pre-transposed bf16 weight tiles streamed from DRAM
    on the sync queue; psum evicted with fused ReLU+bias. Last-layer
    weights/bias stream on the gpsimd queue during layer 2 so they never
    steal sync-queue bandwidth from the weight stream.
  - Last layer swaps matmul args so psum comes out in natural [batch, out]
    layout; bias added on DVE; DMA straight to the output.
  - x1/x2 are cast to bf16 on the host (halves input DMA, removes cast).
"""

import sys

if "/opt/trn_rl_repo" not in sys.path:
    sys.path.insert(0, "/opt/trn_rl_repo")

import numpy as np
import ml_dtypes

import concourse.bass as bass
import concourse.tile as tile
from concourse import bacc, mybir
from concourse.bass_utils import run_bass_kernel_spmd
from concourse.masks import make_identity

P = 128
D = 1024
D2 = 2048
DOUT = 1024
N_LAYERS = 4
N_CORES = 8
B = 16384
BC = B // N_CORES           # rows per core = 2048
BP = BC                     # single pass over the whole core batch
NB_TILES = BC // P          # 16 b-tiles of 128 rows per core
KT = D2 // P                # 16 k tiles (contraction)
MT = D2 // P                # 16 m tiles (layer out features)
KG = 4                      # k-subtiles per weight DMA
NCHUNK = 512                # matmul moving free dim
NQ = 512                    # last-layer o-half width
QT = DOUT // NQ             # 2
NWARM = 28                  # HAM warmup matmuls (N=256, ~6us bridge)

f32 = mybir.dt.float32
bf16 = mybir.dt.bfloat16
NP_BF16 = np.dtype(ml_dtypes.bfloat16)
AF = mybir.ActivationFunctionType
ALU = mybir.AluOpType

# layer-1 runs as block A (cols 0:512, from bootstrap tiles 0-3) with ALL
# remaining attention tiles hooked into its m-loop, then one merged m-loop
# for cols 512:2048 (3 psums per m, ONE weight stream instead of three).
L1A_HOOKS = {0: [("s", 4), ("s", 5)], 1: [("s", 6)],
             2: [("c", 4), ("s", 7)], 3: [("c", 5), ("s", 8)],
             4: [("c", 6), ("s", 9)], 5: [("c", 7), ("s", 10)],
             6: [("c", 8), ("s", 11)], 7: [("c", 9), ("s", 12)],
             8: [("c", 10), ("s", 13)], 9: [("c", 11), ("s", 14)],
             10: [("c", 12), ("s", 15)], 11: [("c", 13)],
             12: [("c", 14)], 13: [("c", 15)]}


def build_graph(debug_stage=None):
    nc = bacc.Bacc("TRN2", target_bir_lowering=False, debug=False,
                   num_devices=N_CORES)

    x1_ext = nc.declare_dram_parameter("x1b", [BC, D], bf16, isOutput=False)
    x2_ext = nc.declare_dram_parameter("x2b", [BC, D], bf16, isOutput=False)
    # weight tiles: [l, m, i(128), k(16), o(128)] bf16, k-axis permuted to
    # chunk-major slots (see prep_inputs); 4KB contiguous per partition so
    # each m-group loads in ONE 512KB DMA
    wt_ext = nc.declare_dram_parameter("wt", [N_LAYERS, MT, P, KT, P],
                                       bf16, isOutput=False)
    # last-layer tiles: [k, i(128), o(1024)] bf16 (k-axis permuted likewise)
    wlt_ext = nc.declare_dram_parameter("wlt", [KT, P, DOUT], bf16,
                                        isOutput=False)
    # biases: bst[l, p, m] = bs[l, m*128+p]
    bst_ext = nc.declare_dram_parameter("bst", [N_LAYERS, P, MT], f32,
                                        isOutput=False)
    # b_last replicated across partitions: [128, 1024]
    blb_ext = nc.declare_dram_parameter("blb", [P, DOUT], bf16, isOutput=False)
    out_ext = nc.declare_dram_parameter("out", [BC, DOUT], f32, isOutput=True)
    dbg_ext = None
    if debug_stage is not None:
        dbg_ext = nc.declare_dram_parameter("dbg", [P, KT // 2, 2, BP], bf16,
                                            isOutput=True)

    with tile.TileContext(nc) as tc:
        _trace(nc, tc, x1_ext, x2_ext, wt_ext, wlt_ext, bst_ext, blb_ext,
               out_ext, debug_stage, dbg_ext)
    nc.compile()
    return nc


def _trace(nc, tc, x1_ext, x2_ext, wt_ext, wlt_ext, bst_ext, blb_ext, out_ext,
           debug_stage=None, dbg_ext=None):
    from contextlib import ExitStack
    ctx = ExitStack()
    with ctx:
        const = ctx.enter_context(tc.tile_pool(name="const", bufs=1))
        acts = ctx.enter_context(tc.tile_pool(name="acts", bufs=2))
        wpool = ctx.enter_context(tc.tile_pool(name="wpool", bufs=4))
        wlpool = ctx.enter_context(tc.tile_pool(name="wlpool", bufs=1))
        cpool = ctx.enter_context(tc.tile_pool(name="cpool", bufs=2))
        x2pool = ctx.enter_context(tc.tile_pool(name="x2pool", bufs=3))
        dlpool = ctx.enter_context(tc.tile_pool(name="dlpool", bufs=3))
        spool = ctx.enter_context(tc.tile_pool(name="spool", bufs=2))
        stpool = ctx.enter_context(tc.tile_pool(name="stpool", bufs=2))
        smpool = ctx.enter_context(tc.tile_pool(name="smpool", bufs=4))
        dpool = ctx.enter_context(tc.tile_pool(name="dpool", bufs=3))
        mpsum = ctx.enter_context(tc.tile_pool(name="mpsum", bufs=8,
                                               space="PSUM"))

        # constants + ACT table preloads (Square first, then Sigmoid) so
        # neither 1.28us ACT_TABLE_LOAD lands on the tile-0 critical path
        warm = const.tile([P, 1], f32)
        nc.vector.memset(warm[:], 0.0)
        nc.scalar.activation(warm[:], warm[:], AF.Square)
        nc.scalar.activation(warm[:], warm[:], AF.Sigmoid)

        ident = const.tile([P, P], f32)
        make_identity(nc, ident)
        idp = const.tile([P, 2 * P], bf16)    # [I | I]
        nc.vector.tensor_copy(idp[:, 0:P], ident[:])
        nc.vector.tensor_copy(idp[:, P:2 * P], ident[:])

        # HAM warmup: small matmuls keep the PE active from t=0 so the
        # clock gate reaches K=8/8 before the first real matmul.
        wz = const.tile([P, P], bf16)
        nc.vector.memset(wz[:], 0.0)
        wps = mpsum.tile([P, NCHUNK], f32, name="mps")
        for i in range(NWARM):
            nc.tensor.matmul(wps[:, 0:2 * P], wz[:], idp[:],
                             start=True, stop=True)

        bst_sb = const.tile([P, N_LAYERS * MT], f32)
        blb_sb = const.tile([P, DOUT], bf16)
        wl_tiles = [wlpool.tile([P, DOUT], bf16, name=f"wl{k}")
                    for k in range(KT)]

        def load_small_consts():
            for l in range(N_LAYERS):
                nc.sync.dma_start(bst_sb[:, l * MT:(l + 1) * MT],
                                  bst_ext.ap()[l])

        def load_wl(m):
            # last-layer weights + blb on the gpsimd queue, pinned late so
            # the scheduler cannot hoist them into the bootstrap where they
            # would steal DMA-fabric bandwidth from the attention x loads
            with tc.tile_wait_until(0.5):
                nc.gpsimd.dma_start(wl_tiles[m][:], wlt_ext.ap()[m])
                if m == MT - 1:
                    nc.gpsimd.dma_start(blb_sb[:], blb_ext.ap()[:, :])

        # ---------- attention ----------
        h0 = acts.tile([P, KT // 2, 2, BP], bf16, name="hbuf")
        attn_state = {}

        def attn_stats(t):
            xc1 = cpool.tile([P, D], bf16, name="xc1")
            nc.gpsimd.dma_start(xc1[:], x1_ext.ap()[t * P:(t + 1) * P, :])
            xc2 = x2pool.tile([P, D], bf16, name="xc2")
            nc.gpsimd.dma_start(xc2[:], x2_ext.ap()[t * P:(t + 1) * P, :])

            dl = dlpool.tile([P, D], bf16, name="dl")
            nc.vector.tensor_sub(dl[:], xc1[:], xc2[:])
            dt_ = smpool.tile([P, 2], f32, name="dt")
            ee = smpool.tile([P, 1], f32, name="ee")
            scr = spool.tile([P, D], bf16, name="scr")
            nc.vector.scalar_tensor_tensor(scr[:], xc1[:], 1.0 / 32.0, dl[:],
                                           ALU.mult, ALU.mult,
                                           accum_out=dt_[:, 0:1])
            scr2 = spool.tile([P, D], bf16, name="scr")
            nc.scalar.activation(scr2[:], dl[:], AF.Square,
                                 scale=float(1.0 / np.sqrt(32.0)),
                                 accum_out=ee[:])
            nc.vector.tensor_sub(dt_[:, 1:2], dt_[:, 0:1], ee[:])
            coef = smpool.tile([P, 2], f32, name="coef")
            nc.scalar.activation(coef[:], dt_[:], AF.Sigmoid)
            # diagA = [diag(a00) | diag(a10)]
            diagA = dpool.tile([P, 2 * P], bf16, name="diagA")
            nc.vector.tensor_scalar_mul(diagA[:, 0:P], ident[:],
                                        coef[:, 0:1])
            nc.vector.tensor_scalar_mul(diagA[:, P:2 * P], ident[:],
                                        coef[:, 1:2])
            attn_state[t] = (xc2, dl, diagA)

        def attn_combine(t):
            xc2, dl, diagA = attn_state.pop(t)
            col = t * P
            for pc in range(D // P // 2):   # 4 chunk pairs
                ps = mpsum.tile([P, NCHUNK], f32, name="mps")
                for hf in range(2):
                    dc = pc * 2 + hf
                    sl = slice(hf * 2 * P, (hf + 1) * 2 * P)
                    nc.tensor.matmul(ps[:, sl], xc2[:, dc * P:(dc + 1) * P],
                                     idp[:], start=True, stop=False)
                    nc.tensor.matmul(ps[:, sl], dl[:, dc * P:(dc + 1) * P],
                                     diagA[:], start=False, stop=True)
                # single [128, 2x2x128] eviction (chunk, tok, col)
                dst = h0[:, 2 * pc:2 * pc + 2, :, col:col + P]
                if pc % 2 == 0:
                    nc.scalar.copy(dst, ps[:])
                else:
                    nc.vector.tensor_copy(dst, ps[:])

        def run_hooks(evs):
            for kind, t in evs:
                if kind == "s":
                    attn_stats(t)
                else:
                    attn_combine(t)

        def layer1_blockA(h_in, h_out):
            # cols 0:512 with attention hooks for tiles 4..15
            for m in range(MT):
                if m in L1A_HOOKS:
                    run_hooks(L1A_HOOKS[m])
                ps = mpsum.tile([P, NCHUNK], f32, name="mps")
                wt = wpool.tile([P, KT, P], bf16, name="wt")
                # keep the weight prefetch off the DMA fabric until the
                # bootstrap x tiles have landed
                with tc.tile_wait_until(0.012):
                    nc.sync.dma_start(wt[:], wt_ext.ap()[0, m])
                for k in range(KT):
                    nc.tensor.matmul(
                        ps[:], wt[:, k, :],
                        h_in[:, k // 2, k % 2, 0:NCHUNK],
                        start=(k == 0), stop=(k == KT - 1))
                # ReLU+bias eviction on ACT (DVE is loaded with attn work)
                nc.scalar.activation(h_out[:, m % 8, m // 8, 0:NCHUNK],
                                     ps[:], AF.Relu,
                                     bias=bst_sb[:, m:m + 1])

        def layer1_rest(h_in, h_out):
            # cols 512:2048: 3 psums per m share one weight stream
            for m in range(MT):
                pss = [mpsum.tile([P, NCHUNK], f32, name="mps")
                       for _ in range(3)]
                wt = wpool.tile([P, KT, P], bf16, name="wt")
                nc.sync.dma_start(wt[:], wt_ext.ap()[0, m])
                for k in range(KT):
                    for n in range(3):
                        lo = NCHUNK + n * NCHUNK
                        nc.tensor.matmul(
                            pss[n][:], wt[:, k, :],
                            h_in[:, k // 2, k % 2, lo:lo + NCHUNK],
                            start=(k == 0), stop=(k == KT - 1))
                for n in range(3):
                    lo = NCHUNK + n * NCHUNK
                    nc.scalar.activation(h_out[:, m % 8, m // 8,
                                               lo:lo + NCHUNK],
                                         pss[n][:], AF.Relu,
                                         bias=bst_sb[:, m:m + 1])

        if debug_stage == "attn":
            load_small_consts()
            for t in range(NB_TILES):
                attn_stats(t)
                attn_combine(t)
            nc.sync.dma_start(dbg_ext.ap()[:, :, :, :], h0[:])
            return

        # bootstrap tiles 0-3 then interleave the rest into layer 1
        load_small_consts()
        attn_stats(0)
        attn_stats(1)
        attn_combine(0)
        attn_stats(2)
        attn_combine(1)
        attn_stats(3)
        attn_combine(2)
        attn_combine(3)
        h1 = acts.tile([P, KT // 2, 2, BP], bf16, name="hbuf")
        layer1_blockA(h0, h1)
        layer1_rest(h0, h1)
        h = h1

        # ---------- MLP layers 2..4 (feature-major) ----------
        for l in range(1, N_LAYERS):
            hout = acts.tile([P, KT // 2, 2, BP], bf16, name="hbuf")
            for m in range(MT):
                pss = [mpsum.tile([P, NCHUNK], f32, name="mps")
                       for _ in range(BP // NCHUNK)]
                wt = wpool.tile([P, KT, P], bf16, name="wt")
                nc.sync.dma_start(wt[:], wt_ext.ap()[l, m])
                for k in range(KT):
                    first = (k == 0)
                    last = (k == KT - 1)
                    for n in range(BP // NCHUNK):
                        nc.tensor.matmul(
                            pss[n][:], wt[:, k, :],
                            h[:, k // 2, k % 2,
                              n * NCHUNK:(n + 1) * NCHUNK],
                            start=first, stop=last)
                bias = bst_sb[:, l * MT + m:l * MT + m + 1]
                for n in range(BP // NCHUNK):
                    nc.scalar.activation(
                        hout[:, m % 8, m // 8, n * NCHUNK:(n + 1) * NCHUNK],
                        pss[n][:], AF.Relu, bias=bias)
                if l == 1:
                    load_wl(m)
            h = hout

        if debug_stage == "mlp":
            nc.sync.dma_start(dbg_ext.ap()[:, :, :, :], h[:])
            return

        # ---------- last layer: natural-layout output ----------
        for m in range(BP // P):  # 16 batch chunks of 128
            pss = [mpsum.tile([P, NCHUNK], f32, name="mps")
                   for _ in range(QT)]
            for k in range(KT):
                for q in range(QT):
                    nc.tensor.matmul(pss[q][:],
                                     h[:, k // 2, k % 2, m * P:(m + 1) * P],
                                     wl_tiles[k][:, q * NQ:(q + 1) * NQ],
                                     start=(k == 0), stop=(k == KT - 1))
            for q in range(QT):
                stg = stpool.tile([P, NQ], f32, name="stg")
                nc.vector.tensor_add(stg[:], pss[q][:],
                                     blb_sb[:, q * NQ:(q + 1) * NQ])
                r0 = m * P
                nc.sync.dma_start(
                    out_ext.ap()[r0:r0 + P, q * NQ:(q + 1) * NQ], stg[:])


# k-slot permutation: slot knew = chunk*2 + tok holds original k-block
# kold = tok*8 + chunk  (feature f = tok*1024 + chunk*128 + i)
KPERM = [(k % 2) * 8 + k // 2 for k in range(KT)]


def prep_inputs(x1, x2, Ws, bs, W_last, b_last):
    """Host-side layout prep shared by all cores (weights) + per-core shards."""
    Wsp = np.ascontiguousarray(
        Ws.reshape(N_LAYERS, D2, KT, P)[:, :, KPERM, :].reshape(
            N_LAYERS, D2, D2))
    Wlp = np.ascontiguousarray(
        W_last.reshape(DOUT, KT, P)[:, KPERM, :].reshape(DOUT, D2))
    # wt[l, m, i, k, o] = Wsp[l, m*128+o, k*128+i]
    wt = np.ascontiguousarray(
        Wsp.reshape(N_LAYERS, MT, P, KT, P)
        .transpose(0, 1, 4, 3, 2)).astype(NP_BF16)
    wlt = np.ascontiguousarray(
        Wlp.reshape(DOUT, KT, P).transpose(1, 2, 0)).astype(NP_BF16)
    bst = np.ascontiguousarray(
        bs.reshape(N_LAYERS, MT, P).transpose(0, 2, 1))
    blb = np.ascontiguousarray(np.broadcast_to(b_last, (P, DOUT))).astype(NP_BF16)
    shared = {"wt": wt, "wlt": wlt, "bst": bst, "blb": blb}
    x1b = x1.astype(NP_BF16)
    x2b = x2.astype(NP_BF16)
    in_maps = []
    for c in range(N_CORES):
        sl = slice(c * BC, (c + 1) * BC)
        m = {"x1b": np.ascontiguousarray(x1b[sl]),
             "x2b": np.ascontiguousarray(x2b[sl])}
        m.update(shared)
        in_maps.append(m)
    return in_maps


_compiled_nc = None


def kernel(x1, x2, Ws, bs, W_last, b_last):
    global _compiled_nc
    x1 = np.asarray(x1, dtype=np.float32)
    x2 = np.asarray(x2, dtype=np.float32)
    Ws = np.asarray(Ws, dtype=np.float32)
    bs = np.asarray(bs, dtype=np.float32)
    W_last = np.asarray(W_last, dtype=np.float32)
    b_last = np.asarray(b_last, dtype=np.float32)

    if _compiled_nc is None:
        _compiled_nc = build_graph()
    in_maps = prep_inputs(x1, x2, Ws, bs, W_last, b_last)
    res = run_bass_kernel_spmd(_compiled_nc, in_maps,
                               core_ids=list(range(N_CORES)))
    out = np.concatenate([res.results[c]["out"] for c in range(N_CORES)],
                         axis=0)
    return out.astype(np.float32)


# revision 46
# speedup vs baseline: 1.0058x; 1.0058x over previous
"""Trainium2 Bass kernel for nn_AndAttention.

B=16384 rows; per row: 2-token self-attention over (x1,x2) [D=1024 each],
concat -> h [2048], then 4x (Linear(2048,2048)+ReLU) and Linear(2048,1024).

Sharding: data-parallel over batch across 8 NeuronCores (2048 rows/core),
weights replicated. No collectives.

Layout strategy (per core):
  - Activations feature-major in SBUF, chunk-major slots: h[P, 8 chunk,
    2 tok, batch]; feature f = tok*1024 + chunk*128 + i lives in slot
    (chunk, tok). Weight k-axes are permuted on the host to match.
  - 2-token softmax via the delta trick: with delta = x1-x2,
      d0 = x1.delta/32 = s11-s12   (DVE stt with accumulate)
      e  = |delta|^2/32            (ACT Square with accumulate)
      d1 = d0 - e = s12-s22
      a00 = sigmoid(d0), a10 = sigmoid(d1)
    and y0 = x2 + a00*delta, y1 = x2 + a10*delta.
  - Attention combine+transpose fused on the PE with only 2 stationary
    operands per chunk (x2 and delta):
      psum[f, 0:256] = x2c.T @ [I|I] + dc.T @ [diag(a00)|diag(a10)]
    Two chunks share one full PSUM bank (2 accumulation groups per bank),
    evicted in a single [128,512] copy per pair (ACT/DVE alternating).
  - Attention tiles are interleaved per-tile into layer 1's m-loop via
    hooks so stats/DMA always overlap PE work and HAM stays warm.
  - Warmup matmuls + ACT table preloads (Square, Sigmoid) at t=0.
  - MLP layers: lhsT = pre-transposed bf16 weight tiles streamed from DRAM
    on the sync queue; psum evicted with fused ReLU+bias. Last-layer
    weights/bias stream on the gpsimd queue during layer 2 so they never
    steal sync-queue bandwidth from the weight stream.
  - Last layer swaps matmul args so psum comes out in natural [batch, out]
    layout; bias added on DVE; DMA straight to the output.
  - x1/x2 are cast to bf16 on the host (halves input DMA, removes cast).
"""

import sys

if "/opt/trn_rl_repo" not in sys.path:
    sys.path.insert(0, "/opt/trn_rl_repo")

import numpy as np
import ml_dtypes

import concourse.bass as bass
import concourse.tile as tile
from concourse import bacc, mybir
from concourse.bass_utils import run_bass_kernel_spmd
from concourse.masks import make_identity

P = 128
D = 1024
D2 = 2048
DOUT = 1024
N_LAYERS = 4
N_CORES = 8
B = 16384
BC = B // N_CORES           # rows per core = 2048
BP = BC                     # single pass over the whole core batch
NB_TILES = BC // P          # 16 b-tiles of 128 rows per core
KT = D2 // P                # 16 k tiles (contraction)
MT = D2 // P                # 16 m tiles (layer out features)
KG = 4                      # k-subtiles per weight DMA
NCHUNK = 512                # matmul moving free dim
NQ = 512                    # last-layer o-half width
QT = DOUT // NQ             # 2
NWARM = 28                  # HAM warmup matmuls (N=256, ~6us bridge)

f32 = mybir.dt.float32
bf16 = mybir.dt.bfloat16
NP_BF16 = np.dtype(ml_dtypes.bfloat16)
AF = mybir.ActivationFunctionType
ALU = mybir.AluOpType

# layer-1 runs as block A (cols 0:512, from bootstrap tiles 0-3) with ALL
# remaining attention tiles hooked into its m-loop, then one merged m-loop
# for cols 512:2048 (3 psums per m, ONE weight stream instead of three).
L1A_HOOKS = {0: [("s", 4), ("s", 5)], 1: [("s", 6)],
             2: [("c", 4), ("s", 7)], 3: [("c", 5), ("s", 8)],
             4: [("c", 6), ("s", 9)], 5: [("c", 7), ("s", 10)],
             6: [("c", 8), ("s", 11)], 7: [("c", 9), ("s", 12)],
             8: [("c", 10), ("s", 13)], 9: [("c", 11), ("s", 14)],
             10: [("c", 12), ("s", 15)], 11: [("c", 13)],
             12: [("c", 14)], 13: [("c", 15)]}


def build_graph(debug_stage=None):
    nc = bacc.Bacc("TRN2", target_bir_lowering=False, debug=False,
                   num_devices=N_CORES)

    x1_ext = nc.declare_dram_parameter("x1b", [BC, D], bf16, isOutput=False)
    x2_ext = nc.declare_dram_parameter("x2b", [BC, D], bf16, isOutput=False)
    # weight tiles: [l, m, i(128), k(16), o(128)] bf16, k-axis permuted to
    # chunk-major slots (see prep_inputs); 4KB contiguous per partition so
    # each m-group loads in ONE 512KB DMA
    wt_ext = nc.declare_dram_parameter("wt", [N_LAYERS, MT, P, KT, P],
                                       bf16, isOutput=False)
    # last-layer tiles: [k, i(128), o(1024)] bf16 (k-axis permuted likewise)
    wlt_ext = nc.declare_dram_parameter("wlt", [KT, P, DOUT], bf16,
                                        isOutput=False)
    # biases: bst[l, p, m] = bs[l, m*128+p]
    bst_ext = nc.declare_dram_parameter("bst", [N_LAYERS, P, MT], f32,
                                        isOutput=False)
    # b_last replicated across partitions: [128, 1024]
    blb_ext = nc.declare_dram_parameter("blb", [P, DOUT], bf16, isOutput=False)
    out_ext = nc.declare_dram_parameter("out", [BC, DOUT], f32, isOutput=True)
    dbg_ext = None
    if debug_stage is not None:
        dbg_ext = nc.declare_dram_parameter("dbg", [P, KT // 2, 2, BP], bf16,
                                            isOutput=True)

    with tile.TileContext(nc) as tc:
        _trace(nc, tc, x1_ext, x2_ext, wt_ext, wlt_ext, bst_ext, blb_ext,
               out_ext, debug_stage, dbg_ext)
    nc.compile()
    return nc


def _trace(nc, tc, x1_ext, x2_ext, wt_ext, wlt_ext, bst_ext, blb_ext, out_ext,
           debug_stage=None, dbg_ext=None):
    from contextlib import ExitStack
    ctx = ExitStack()
    with ctx:
        const = ctx.enter_context(tc.tile_pool(name="const", bufs=1))
        acts = ctx.enter_context(tc.tile_pool(name="acts", bufs=2))
        wpool = ctx.enter_context(tc.tile_pool(name="wpool", bufs=4))
        wlpool = ctx.enter_context(tc.tile_pool(name="wlpool", bufs=1))
        cpool = ctx.enter_context(tc.tile_pool(name="cpool", bufs=2))
        x2pool = ctx.enter_context(tc.tile_pool(name="x2pool", bufs=3))
        dlpool = ctx.enter_context(tc.tile_pool(name="dlpool", bufs=3))
        spool = ctx.enter_context(tc.tile_pool(name="spool", bufs=2))
        stpool = ctx.enter_context(tc.tile_pool(name="stpool", bufs=2))
        smpool = ctx.enter_context(tc.tile_pool(name="smpool", bufs=4))
        dpool = ctx.enter_context(tc.tile_pool(name="dpool", bufs=3))
        mpsum = ctx.enter_context(tc.tile_pool(name="mpsum", bufs=8,
                                               space="PSUM"))

        # constants + ACT table preloads (Square first, then Sigmoid) so
        # neither 1.28us ACT_TABLE_LOAD lands on the tile-0 critical path
        warm = const.tile([P, 1], f32)
        nc.vector.memset(warm[:], 0.0)
        nc.scalar.activation(warm[:], warm[:], AF.Square)
        nc.scalar.activation(warm[:], warm[:], AF.Sigmoid)

        ident = const.tile([P, P], f32)
        make_identity(nc, ident)
        idp = const.tile([P, 2 * P], bf16)    # [I | I]
        nc.vector.tensor_copy(idp[:, 0:P], ident[:])
        nc.vector.tensor_copy(idp[:, P:2 * P], ident[:])

        # HAM warmup: small matmuls keep the PE active from t=0 so the
        # clock gate reaches K=8/8 before the first real matmul.
        wz = const.tile([P, P], bf16)
        nc.vector.memset(wz[:], 0.0)
        wps = mpsum.tile([P, NCHUNK], f32, name="mps")
        for i in range(NWARM):
            nc.tensor.matmul(wps[:, 0:2 * P], wz[:], idp[:],
                             start=True, stop=True)

        bst_sb = const.tile([P, N_LAYERS * MT], f32)
        blb_sb = const.tile([P, DOUT], bf16)
        wl_tiles = [wlpool.tile([P, DOUT], bf16, name=f"wl{k}")
                    for k in range(KT)]

        def load_small_consts():
            for l in range(N_LAYERS):
                nc.sync.dma_start(bst_sb[:, l * MT:(l + 1) * MT],
                                  bst_ext.ap()[l])

        def load_wl(m):
            # last-layer weights + blb on the gpsimd queue, pinned late so
            # the scheduler cannot hoist them into the bootstrap where they
            # would steal DMA-fabric bandwidth from the attention x loads
            with tc.tile_wait_until(0.5):
                nc.gpsimd.dma_start(wl_tiles[m][:], wlt_ext.ap()[m])
                if m == MT - 1:
                    nc.gpsimd.dma_start(blb_sb[:], blb_ext.ap()[:, :])

        # ---------- attention ----------
        h0 = acts.tile([P, KT // 2, 2, BP], bf16, name="hbuf")
        attn_state = {}

        def attn_stats(t):
            xc1 = cpool.tile([P, D], bf16, name="xc1")
            nc.gpsimd.dma_start(xc1[:], x1_ext.ap()[t * P:(t + 1) * P, :])
            xc2 = x2pool.tile([P, D], bf16, name="xc2")
            nc.gpsimd.dma_start(xc2[:], x2_ext.ap()[t * P:(t + 1) * P, :])

            dl = dlpool.tile([P, D], bf16, name="dl")
            nc.vector.tensor_sub(dl[:], xc1[:], xc2[:])
            dt_ = smpool.tile([P, 2], f32, name="dt")
            ee = smpool.tile([P, 1], f32, name="ee")
            scr = spool.tile([P, D], bf16, name="scr")
            nc.vector.scalar_tensor_tensor(scr[:], xc1[:], 1.0 / 32.0, dl[:],
                                           ALU.mult, ALU.mult,
                                           accum_out=dt_[:, 0:1])
            scr2 = spool.tile([P, D], bf16, name="scr")
            nc.scalar.activation(scr2[:], dl[:], AF.Square,
                                 scale=float(1.0 / np.sqrt(32.0)),
                                 accum_out=ee[:])
            nc.vector.tensor_sub(dt_[:, 1:2], dt_[:, 0:1], ee[:])
            coef = smpool.tile([P, 2], f32, name="coef")
            nc.scalar.activation(coef[:], dt_[:], AF.Sigmoid)
            # diagA = [diag(a00) | diag(a10)]
            diagA = dpool.tile([P, 2 * P], bf16, name="diagA")
            nc.vector.tensor_scalar_mul(diagA[:, 0:P], ident[:],
                                        coef[:, 0:1])
            nc.vector.tensor_scalar_mul(diagA[:, P:2 * P], ident[:],
                                        coef[:, 1:2])
            attn_state[t] = (xc2, dl, diagA)

        def attn_combine(t):
            xc2, dl, diagA = attn_state.pop(t)
            col = t * P
            for pc in range(D // P // 2):   # 4 chunk pairs
                ps = mpsum.tile([P, NCHUNK], f32, name="mps")
                for hf in range(2):
                    dc = pc * 2 + hf
                    sl = slice(hf * 2 * P, (hf + 1) * 2 * P)
                    nc.tensor.matmul(ps[:, sl], xc2[:, dc * P:(dc + 1) * P],
                                     idp[:], start=True, stop=False)
                    nc.tensor.matmul(ps[:, sl], dl[:, dc * P:(dc + 1) * P],
                                     diagA[:], start=False, stop=True)
                # single [128, 2x2x128] eviction (chunk, tok, col)
                dst = h0[:, 2 * pc:2 * pc + 2, :, col:col + P]
                if pc % 2 == 0:
                    nc.scalar.copy(dst, ps[:])
                else:
                    nc.vector.tensor_copy(dst, ps[:])

        def run_hooks(evs):
            for kind, t in evs:
                if kind == "s":
                    attn_stats(t)
                else:
                    attn_combine(t)

        def layer1_blockA(h_in, h_out):
            # cols 0:512 with attention hooks for tiles 4..15
            for m in range(MT):
                if m in L1A_HOOKS:
                    run_hooks(L1A_HOOKS[m])
                ps = mpsum.tile([P, NCHUNK], f32, name="mps")
                wt = wpool.tile([P, KT, P], bf16, name="wt")
                # keep the weight prefetch off the DMA fabric until the
                # bootstrap x tiles have landed
                with tc.tile_wait_until(0.012):
                    nc.sync.dma_start(wt[:], wt_ext.ap()[0, m])
                for k in range(KT):
                    nc.tensor.matmul(
                        ps[:], wt[:, k, :],
                        h_in[:, k // 2, k % 2, 0:NCHUNK],
                        start=(k == 0), stop=(k == KT - 1))
                # ReLU+bias eviction on ACT (DVE is loaded with attn work)
                nc.scalar.activation(h_out[:, m % 8, m // 8, 0:NCHUNK],
                                     ps[:], AF.Relu,
                                     bias=bst_sb[:, m:m + 1])

        def layer1_rest(h_in, h_out):
            # cols 512:2048: 3 psums per m share one weight stream.
            # n-OUTER so each psum's accumulation finishes mid-group and
            # its eviction pipelines instead of clustering at the boundary.
            for m in range(MT):
                wt = wpool.tile([P, KT, P], bf16, name="wt")
                nc.sync.dma_start(wt[:], wt_ext.ap()[0, m])
                for n in range(3):
                    lo = NCHUNK + n * NCHUNK
                    ps = mpsum.tile([P, NCHUNK], f32, name="mps")
                    for k in range(KT):
                        nc.tensor.matmul(
                            ps[:], wt[:, k, :],
                            h_in[:, k // 2, k % 2, lo:lo + NCHUNK],
                            start=(k == 0), stop=(k == KT - 1))
                    nc.scalar.activation(h_out[:, m % 8, m // 8,
                                               lo:lo + NCHUNK],
                                         ps[:], AF.Relu,
                                         bias=bst_sb[:, m:m + 1])

        if debug_stage == "attn":
            load_small_consts()
            for t in range(NB_TILES):
                attn_stats(t)
                attn_combine(t)
            nc.sync.dma_start(dbg_ext.ap()[:, :, :, :], h0[:])
            return

        # bootstrap tiles 0-3 then interleave the rest into layer 1
        load_small_consts()
        attn_stats(0)
        attn_stats(1)
        attn_combine(0)
        attn_stats(2)
        attn_combine(1)
        attn_stats(3)
        attn_combine(2)
        attn_combine(3)
        h1 = acts.tile([P, KT // 2, 2, BP], bf16, name="hbuf")
        layer1_blockA(h0, h1)
        layer1_rest(h0, h1)
        h = h1

        # ---------- MLP layers 2..4 (feature-major) ----------
        for l in range(1, N_LAYERS):
            hout = acts.tile([P, KT // 2, 2, BP], bf16, name="hbuf")
            for m in range(MT):
                wt = wpool.tile([P, KT, P], bf16, name="wt")
                nc.sync.dma_start(wt[:], wt_ext.ap()[l, m])
                bias = bst_sb[:, l * MT + m:l * MT + m + 1]
                for n in range(BP // NCHUNK):
                    ps = mpsum.tile([P, NCHUNK], f32, name="mps")
                    for k in range(KT):
                        nc.tensor.matmul(
                            ps[:], wt[:, k, :],
                            h[:, k // 2, k % 2,
                              n * NCHUNK:(n + 1) * NCHUNK],
                            start=(k == 0), stop=(k == KT - 1))
                    nc.scalar.activation(
                        hout[:, m % 8, m // 8, n * NCHUNK:(n + 1) * NCHUNK],
                        ps[:], AF.Relu, bias=bias)
                if l == 1:
                    load_wl(m)
            h = hout

        if debug_stage == "mlp":
            nc.sync.dma_start(dbg_ext.ap()[:, :, :, :], h[:])
            return

        # ---------- last layer: natural-layout output ----------
        for m in range(BP // P):  # 16 batch chunks of 128
            for q in range(QT):
                ps = mpsum.tile([P, NCHUNK], f32, name="mps")
                for k in range(KT):
                    nc.tensor.matmul(ps[:],
                                     h[:, k // 2, k % 2, m * P:(m + 1) * P],
                                     wl_tiles[k][:, q * NQ:(q + 1) * NQ],
                                     start=(k == 0), stop=(k == KT - 1))
                stg = stpool.tile([P, NQ], f32, name="stg")
                nc.vector.tensor_add(stg[:], ps[:],
                                     blb_sb[:, q * NQ:(q + 1) * NQ])
                r0 = m * P
                nc.sync.dma_start(
                    out_ext.ap()[r0:r0 + P, q * NQ:(q + 1) * NQ], stg[:])


# k-slot permutation: slot knew = chunk*2 + tok holds original k-block
# kold = tok*8 + chunk  (feature f = tok*1024 + chunk*128 + i)
KPERM = [(k % 2) * 8 + k // 2 for k in range(KT)]


def prep_inputs(x1, x2, Ws, bs, W_last, b_last):
    """Host-side layout prep shared by all cores (weights) + per-core shards."""
    Wsp = np.ascontiguousarray(
        Ws.reshape(N_LAYERS, D2, KT, P)[:, :, KPERM, :].reshape(
            N_LAYERS, D2, D2))
    Wlp = np.ascontiguousarray(
        W_last.reshape(DOUT, KT, P)[:, KPERM, :].reshape(DOUT, D2))
    # wt[l, m, i, k, o] = Wsp[l, m*128+o, k*128+i]
    wt = np.ascontiguousarray(
        Wsp.reshape(N_LAYERS, MT, P, KT, P)
        .transpose(0, 1, 4, 3, 2)).astype(NP_BF16)
    wlt = np.ascontiguousarray(
        Wlp.reshape(DOUT, KT, P).transpose(1, 2, 0)).astype(NP_BF16)
    bst = np.ascontiguousarray(
        bs.reshape(N_LAYERS, MT, P).transpose(0, 2, 1))
    blb = np.ascontiguousarray(np.broadcast_to(b_last, (P, DOUT))).astype(NP_BF16)
    shared = {"wt": wt, "wlt": wlt, "bst": bst, "blb": blb}
    x1b = x1.astype(NP_BF16)
    x2b = x2.astype(NP_BF16)
    in_maps = []
    for c in range(N_CORES):
        sl = slice(c * BC, (c + 1) * BC)
        m = {"x1b": np.ascontiguousarray(x1b[sl]),
             "x2b": np.ascontiguousarray(x2b[sl])}
        m.update(shared)
        in_maps.append(m)
    return in_maps


_compiled_nc = None


def kernel(x1, x2, Ws, bs, W_last, b_last):
    global _compiled_nc
    x1 = np.asarray(x1, dtype=np.float32)
    x2 = np.asarray(x2, dtype=np.float32)
    Ws = np.asarray(Ws, dtype=np.float32)
    bs = np.asarray(bs, dtype=np.float32)
    W_last = np.asarray(W_last, dtype=np.float32)
    b_last = np.asarray(b_last, dtype=np.float32)

    if _compiled_nc is None:
        _compiled_nc = build_graph()
    in_maps = prep_inputs(x1, x2, Ws, bs, W_last, b_last)
    res = run_bass_kernel_spmd(_compiled_nc, in_maps,
                               core_ids=list(range(N_CORES)))
    out = np.concatenate([res.results[c]["out"] for c in range(N_CORES)],
                         axis=0)
    return out.astype(np.float32)
